# revision 20
# baseline (speedup 1.0000x reference)
"""Trainium2 Bass kernel for nn_DenseSum_28698971471971.

Math (per (scope, decomp) pair, 256 of them, all independent):
    log_weights = log_softmax(log(acc), axis=i)
    out[b, j]   = logsumexp_i(x[b, i] + log_weights[i, j])
                = log(sum_i exp(x[b, i]) * acc[i, j]) - log(sum_i acc[i, j])

No max-subtraction needed: x ~ N(0,1) so exp(x) in [e^-6, e^6], acc in
[1e-3, 1]; every sum fits comfortably in fp32.

Numerics: all device I/O is fp16 (e5m10).  |x| <= ~5.5 and acc, exp(x),
and the outputs are all well inside fp16 range; a host-side simulation
of this exact quantization gives max rel err 1.4e-3 vs the fp32
reference (tolerance 2e-2).  fp16 halves DMA bytes vs fp32 -- the
baseline was DMA-bound (70us of DMA_ENGINES time out of 73us).

Layout/algorithm (per core: 32 pairs = 4 scopes x 8 decomps):
  - The host pre-transposes x to x^T[p, i, b] so the contraction dim i
    lands on SBUF partitions with a plain DMA; no PE transposes at all.
  - GEMM computes the TRANSPOSED output y^T[j, b] = acc^T @ exp(x^T):
    stationary = acc[i, j] tiles (natural layout), moving = exp(x^T).
  - The moving operand carries a 257th column of ones, so each matmul
    also accumulates column 256 = sum_i acc[i, j] = the log_softmax
    denominator, landing per j-partition.  Zero extra PE/ACT cost.
  - One batched ACT Ln pass over [y^T | asum] per 2-pair PSUM group
    (both need Ln; group = one [128, 2, 2, 512] f32 tile = 4 of the 8
    PSUM banks, double-buffered).
  - out^T[j, b] = ln_y[j, b] - ln_asum[j] is a per-PARTITION scalar
    subtract (tensor_scalar_sub on DVE).
  - The host un-transposes the [p, j, b] result to [p, b, j] (free).

Scheduling (wall 40.96us = head 3.60 + ACT span 32.83 + tail 4.53):
  - ACT is saturated end-to-end, so the ACT queue order IS the
    schedule: exps for superblock sb+1 are software-pipelined into the
    gaps between superblock sb's Lns (injection after groups 2 and 3),
    which also gives the 2-buffer PSUM ping-pong time to recycle.
  - ACT busy 32.2us = 27.4us elements (exp 16384 + ln 16448 per-
    partition elems at 1.2GHz, ACT-only ops) + 26 ops x 185ns (SBUF
    access overhead) ; table load 1.28us hides in the head shadow.
  - sb0 load order x0,a0,x1,x2,x3,a1,a2,a3 (head_order): front-loading
    x closes ACT's early exp-starvation gaps (-418ns vs x/a 1:1).
  - DMA queues: loads on SP (all emitted up front; they self-throttle
    on tile-buffer semaphores), stores on the Pool SWDGE queue -- a DMA
    holds its queue's SEQ while it waits, so stores need a queue with
    nothing behind them.  The last superblock's stores switch to the
    then-idle SP queue (HWDGE launches ~0.5us faster; that latency is
    the program tail).
  - outf is 8-deep: each store's completion semaphore takes ~0.9us to
    propagate, and with shallow buffering that recycle latency paced
    the whole sub->store tail at 1.17us/group.

Measured floors / dead ends (TimelineSim + real-run evidence; don't
re-explore without new leverage):
  - DMA_ENGINES is ONE serialized resource in the cost model (verified
    by micro-bench: 2 DMAs on different queues take 2x one DMA's
    time).  12.58MB/core fp16 / 360GB/s = 34.9us hard floor.  fp8 on
    x and/or acc fails correctness (max rel 4.7e-2..1.5e-1 vs the
    2e-2 gate; min|expected| = 0.024 leaves ~5e-4 abs budget).
  - exp/ln run ONLY on ACT (1 elem/cycle/partition @ 1.2GHz,
    dtype-independent); DVE/Pool have no LUT ops, poly approximations
    cost >1ns/elem.  ACT element floor 27.4us is irreducible.
  - PSUM matmul accumulation is BANK-granular (start=True zeroing):
    packing (p,jt) y-regions at 257-f32 stride corrupts results
    (rel err 27 on the real path) -- 512-f32 stride is mandatory, so
    >2 pairs per double-buffered Ln batch cannot fit (6+6 > 8 banks).
  - louts must stay f32: ln_y and ln_asum (|.|~5-7) cancel to
    |out|<=1.26; quantizing them to f16 before the subtract gives max
    rel 5.7e-2 (fails).
  - Loads on the ACT HWDGE queue serialize (a DMA holds its queue's
    SEQ until the transfer completes) -> +15us.  Each SP DMA instr
    costs 565ns SEQ time, so finer load chunking always loses.
  - tail_g / per-pair-split last group / merged stores all lose: each
    extra ACT op costs 185ns and each extra store ~200-350ns in the
    final cross-queue drain cascade, >= the latency they'd hide.
  - Tail structure (after last Ln): sem 0.22 + 4 subs 0.78 + HWDGE
    launch 1.36 + transfer 0.73 + DMA sem 0.90 + drain cascade 0.54.
    All hw-spec constants; no schedule fix found.
"""

import numpy as np
from contextlib import ExitStack

import bass_rust as _bass_rust

import concourse.mybir as mybir
import concourse.tile as tile
from concourse import bacc
from concourse.bass_utils import run_bass_kernel_spmd
from concourse.hw_specs import get_activation_tables

F16 = mybir.dt.float16
F32 = mybir.dt.float32
AF = mybir.ActivationFunctionType

NUM_SCOPES, NUM_DECOMPS, BATCH, NUM_IN, NUM_SUMS = 32, 8, 256, 256, 256
N_CORES = 8
SCOPES_PER_CORE = NUM_SCOPES // N_CORES          # 4
PAIRS_PER_CORE = SCOPES_PER_CORE * NUM_DECOMPS   # 32


_HEAD_ORDER_BEST = [("x", 0), ("a", 0), ("x", 1), ("x", 2),
                    ("x", 3), ("a", 1), ("a", 2), ("a", 3)]


def emit_densesum(tc, x_ap, a_ap, o_ap, pairs, head=None, exp_steady=4,
                  tail_g=None, inject=(2, 3), exp_plan=None,
                  ln_inplace=False, split_last=False,
                  head_order=_HEAD_ORDER_BEST,
                  stores_merge=False, head_acc=None,
                  bufs=None, ch=4, group_plan=None, split_x0_it=False,
                  tail_sub_pool=False, steady_order="xaxa"):
    """Emit the kernel body into TileContext `tc`.

    x_ap: [pairs, 256(i), 256(b)] DRAM fp16   (x pre-transposed on host)
    a_ap: [pairs, 256(i), 256(j)] DRAM fp16
    o_ap: [pairs, 256(j), 256(b)] DRAM fp16   (host un-transposes)
    """
    nc = tc.nc
    bufs = bufs or {}
    SB = 8                      # pairs per superblock (exp/x-tile granularity)
    CH = ch                     # pairs per DMA load chunk
    G = 2                       # pairs per PSUM group (Ln granularity)
    assert pairs % SB == 0

    with ExitStack() as ctx:
        ep = ctx.enter_context

        xs_pool = ep(tc.tile_pool(name="xs", bufs=bufs.get("xs", 4)))
        acc_pool = ep(tc.tile_pool(name="accs", bufs=bufs.get("accs", 10)))
        ext_pool = ep(tc.tile_pool(name="ext", bufs=bufs.get("ext", 2)))
        louts_pool = ep(tc.tile_pool(name="louts", bufs=bufs.get("louts", 6)))
        outf_pool = ep(tc.tile_pool(name="outf", bufs=bufs.get("outf", 8)))
        y_pool = ep(tc.tile_pool(name="y", bufs=2, space="PSUM"))

        n_sb = pairs // SB

        # ---- phase 1: emit every load up front (SP queue only).  Each
        # DMA self-throttles on its tile buffer's semaphore, so emission
        # order here just fixes the SP queue order.  x and acc chunks are
        # interleaved so the first matmul group has both inputs ASAP; the
        # first superblock uses small chunks to shorten the pipeline head.
        xss, accss = [], []
        for sb in range(n_sb):
            p0 = sb * SB
            chunks = (head or [2, 2, 2, 2]) if sb == 0 else [CH] * (SB // CH)
            xs = xs_pool.tile([128, SB, 2, 256], F16, name=f"xs{sb}", tag="xs")
            accs = {}

            def load_x(cp, ch):
                nc.sync.dma_start(
                    xs[:, cp:cp + ch, :, :],
                    x_ap[p0 + cp:p0 + cp + ch].rearrange(
                        "p (it i) b -> i p it b", i=128
                    ),
                )

            def load_a(cp, ch, c):
                acc_t = acc_pool.tile(
                    [128, ch, 2, 256], F16, name=f"acc{sb}_{c}", tag="acc"
                )
                nc.sync.dma_start(
                    acc_t[:],
                    a_ap[p0 + cp:p0 + cp + ch].rearrange(
                        "p (it i) j -> i p it j", i=128
                    ),
                )
                for p in range(ch):
                    accs[cp + p] = (acc_t, p)

            if sb == 0 and head_order is not None:
                # head_order: sequence of ('x'|'a', chunk_idx); x uses
                # `chunks`, acc uses `head_acc` sizes (defaults to `chunks`)
                achunks = head_acc or chunks
                offs = [0]
                for ch in chunks:
                    offs.append(offs[-1] + ch)
                aoffs = [0]
                for ch in achunks:
                    aoffs.append(aoffs[-1] + ch)
                for kind, c in head_order:
                    if kind == "x":
                        if c == 0 and split_x0_it:
                            src = x_ap[p0:p0 + chunks[0]].rearrange(
                                "p (it i) b -> i p it b", i=128)
                            for it in range(2):
                                nc.sync.dma_start(
                                    xs[:, 0:chunks[0], it:it + 1, :],
                                    src[:, :, it:it + 1, :])
                        else:
                            load_x(offs[c], chunks[c])
                    else:
                        load_a(aoffs[c], achunks[c], c)
            elif sb > 0 and steady_order == "xxaa":
                cp = 0
                for c, ch in enumerate(chunks):
                    load_x(cp, ch)
                    cp += ch
                cp = 0
                for c, ch in enumerate(chunks):
                    load_a(cp, ch, c)
                    cp += ch
            else:
                cp = 0
                for c, ch in enumerate(chunks):
                    load_x(cp, ch)
                    load_a(cp, ch, c)
                    cp += ch
            xss.append(xs)
            accss.append(accs)

        # ---- phase 2: software-pipelined compute emission.
        # EXT = exp(x^T) plus a 257th column of ones (for asum).  exp
        # chunks for superblock sb+1 are injected BETWEEN the Lns of
        # superblock sb, right where the 2-buffer PSUM ping-pong would
        # otherwise stall the in-order ACT queue (the ~1.9us exp gives
        # the PSUM tiles time to recycle through PE).
        exts = [None] * n_sb

        def emit_exps(sb, ecs):
            ext = ext_pool.tile([128, SB, 2, 257], F16, name=f"ext{sb}", tag="ext")
            exts[sb] = ext
            nc.vector.memset(ext[:, :, :, 256:257], 1.0)
            cp = 0
            for ec in ecs:
                nc.scalar.activation(
                    ext[:, cp:cp + ec, :, 0:256],
                    xss[sb][:, cp:cp + ec, :, :],
                    AF.Exp,
                )
                cp += ec

        # superblock 0: only the first two exp chunks go in front; its
        # last two are injected between its own first Lns (below), filling
        # the ACT-queue gaps where exps would wait on load DMAs and letting
        # the Ln stream start ~2us earlier.
        h = head or [2, 2, 2, 2]
        if split_x0_it:
            ext0 = ext_pool.tile([128, SB, 2, 257], F16, name="ext0", tag="ext")
            exts[0] = ext0
            nc.vector.memset(ext0[:, :, :, 256:257], 1.0)
            for it in range(2):
                nc.scalar.activation(
                    ext0[:, 0:h[0], it:it + 1, 0:256],
                    xss[0][:, 0:h[0], it:it + 1, :], AF.Exp)
            cp1 = h[0]
            for ec in h[1:2]:
                nc.scalar.activation(
                    ext0[:, cp1:cp1 + ec, :, 0:256],
                    xss[0][:, cp1:cp1 + ec, :, :], AF.Exp)
                cp1 += ec
        else:
            emit_exps(0, h[:2])
        sb0_rest = []
        cp0 = h[0] + h[1]
        for ec in h[2:]:
            sb0_rest.append((cp0, cp0 + ec))
            cp0 += ec

        if group_plan is not None:
            # ---- flat-batch path: groups of `group_plan` pairs, PACKED
            # psum tiles ([128, bn, 2, 257] f32), Ln per batch.  Exp chunks
            # are injected after the first batch whose cumulative pair
            # count crosses each threshold (mirrors the sb-loop schedule).
            assert sum(group_plan) == pairs
            accs_flat = {}
            for sb in range(n_sb):
                for p, v in accss[sb].items():
                    accs_flat[sb * SB + p] = v
            # (threshold, sb, lo, hi) exp injections; sb0's deferred rest
            pend = [(2, 0, 4, 6), (4, 0, 6, 8)]
            for sb in range(n_sb - 1):
                pend.append((SB * sb + 6, sb + 1, 0, 4))
                pend.append((SB * sb + 8, sb + 1, 4, 8))
            pend.sort()
            cum = 0
            for b, bn in enumerate(group_plan):
                pb = cum          # first pair of batch
                y = y_pool.tile([128, bn, 2, 257], F32)
                for p in range(bn):
                    acc_t, ac = accs_flat[pb + p]
                    ext = exts[(pb + p) // SB]
                    for jt in range(2):
                        for it in range(2):
                            nc.tensor.matmul(
                                y[:, p, jt, 0:257],
                                acc_t[:, ac, it, jt * 128:(jt + 1) * 128],
                                ext[:, (pb + p) % SB, it, 0:257],
                                start=(it == 0),
                                stop=(it == 1),
                            )
                louts = louts_pool.tile([128, bn, 2, 257], F32)
                nc.scalar.activation(louts[:], y[:], AF.Ln)
                cum += bn
                while pend and pend[0][0] <= cum:
                    _, esb, lo, hi = pend.pop(0)
                    if exts[esb] is None:
                        ext_t = ext_pool.tile([128, SB, 2, 257], F16,
                                              name=f"ext{esb}", tag="ext")
                        exts[esb] = ext_t
                        nc.vector.memset(ext_t[:, :, :, 256:257], 1.0)
                    nc.scalar.activation(
                        exts[esb][:, lo:hi, :, 0:256],
                        xss[esb][:, lo:hi, :, :],
                        AF.Exp,
                    )
                outf = outf_pool.tile([128, bn, 2, 256], F16)
                for p in range(bn):
                    for jt in range(2):
                        nc.vector.tensor_scalar_sub(
                            outf[:, p, jt, :],
                            louts[:, p, jt, 0:256],
                            louts[:, p, jt, 256:257],
                        )
                eng = nc.sync if pb >= pairs - SB else nc.gpsimd
                eng.dma_start(
                    o_ap[pb:pb + bn].rearrange(
                        "p (jt j) b -> j p jt b", j=128),
                    outf[:],
                )
            return

        for sb in range(n_sb):
            p0 = sb * SB
            accs = accss[sb]
            ext = exts[sb]
            last = sb == n_sb - 1
            if last and tail_g == 1:
                groups = [G] * (SB // G - 1) + [1, 1]
            elif last and tail_g == 0:
                groups = [1] * SB
            else:
                groups = [G] * (SB // G)
            gp = 0
            for g, gn in enumerate(groups):
                # y[j_l, p, jt, 0:257] = [y^T | asum], psum f32
                # 512-stride keeps each (p, jt) group bank-aligned
                y = y_pool.tile([128, gn, 2, 512], F32)
                for p in range(gn):
                    acc_t, ac = accs[gp + p]
                    for jt in range(2):
                        for it in range(2):
                            nc.tensor.matmul(
                                y[:, p, jt, 0:257],
                                acc_t[:, ac, it, jt * 128:(jt + 1) * 128],
                                ext[:, gp + p, it, 0:257],
                                start=(it == 0),
                                stop=(it == 1),
                            )
                # ln over the whole [y^T | asum] block in one ACT op
                do_split = split_last and last and g == len(groups) - 1
                if ln_inplace:
                    louts = y
                    if not do_split:
                        nc.scalar.activation(
                            y[:, :, :, 0:257], y[:, :, :, 0:257], AF.Ln)
                else:
                    louts = louts_pool.tile([128, gn, 2, 257], F32)
                    if not do_split:
                        nc.scalar.activation(louts[:], y[:, :, :, 0:257], AF.Ln)
                if do_split:
                    # final group: per-pair Ln -> sub -> store pipeline to
                    # shorten the exposed chain after the very last Ln
                    for p in range(gn):
                        nc.scalar.activation(
                            louts[:, p, :, 0:257], y[:, p, :, 0:257], AF.Ln)
                        outf_p = outf_pool.tile([128, 1, 2, 256], F16)
                        for jt in range(2):
                            nc.vector.tensor_scalar_sub(
                                outf_p[:, 0, jt, :],
                                louts[:, p, jt, 0:256],
                                louts[:, p, jt, 256:257],
                            )
                        nc.sync.dma_start(
                            o_ap[p0 + gp + p:p0 + gp + p + 1].rearrange(
                                "p (jt j) b -> j p jt b", j=128),
                            outf_p[:],
                        )
                    gp += gn
                    continue
                # inject deferred/next-superblock exps between Lns
                if sb == 0 and g < len(sb0_rest):
                    lo0, hi0 = sb0_rest[g]
                    nc.scalar.activation(
                        ext[:, lo0:hi0, :, 0:256],
                        xss[0][:, lo0:hi0, :, :],
                        AF.Exp,
                    )
                if not last:
                    plan = (exp_plan if exp_plan is not None
                            else [(inject[c], c * (SB // len(inject)),
                                   (c + 1) * (SB // len(inject)))
                                  for c in range(len(inject))])
                    chunks_here = [(lo, hi) for (gg, lo, hi) in plan if gg == g]
                    if chunks_here and exts[sb + 1] is None:
                        emit_exps_partial = ext_pool.tile(
                            [128, SB, 2, 257], F16, name=f"ext{sb + 1}", tag="ext"
                        )
                        exts[sb + 1] = emit_exps_partial
                        nc.vector.memset(emit_exps_partial[:, :, :, 256:257], 1.0)
                    for lo, hi in chunks_here:
                        nc.scalar.activation(
                            exts[sb + 1][:, lo:hi, :, 0:256],
                            xss[sb + 1][:, lo:hi, :, :],
                            AF.Exp,
                        )
                # out^T = ln_y - ln_asum (per-partition scalar), on DVE
                # (Pool's 95ns Q7 launch + serial execution made it the
                # store-gating straggler when it handled half of these)
                if stores_merge:
                    if g % 2 == 0:
                        outf = outf_pool.tile([128, 2 * gn, 2, 256], F16)
                        ofs = 0
                    else:
                        ofs = gn
                else:
                    outf = outf_pool.tile([128, gn, 2, 256], F16)
                    ofs = 0
                for p in range(gn):
                    for jt in range(2):
                        nc.vector.tensor_scalar_sub(
                            outf[:, ofs + p, jt, :],
                            louts[:, p, jt, 0:256],
                            louts[:, p, jt, 256:257],
                        )
                # store via the Pool SWDGE queue: a DMA holds its queue's
                # SEQ while waiting, so stores get a queue of their own
                # (on ACT they blocked Ln decode; on SP they'd block
                # loads).  Final superblock: loads are done, SP is free,
                # and HWDGE launch latency (~1.3us) beats SWDGE's (~1.8us)
                # -- that latency is the program's tail.
                eng = nc.sync if last else nc.gpsimd
                if stores_merge:
                    if g % 2 == 1:
                        eng.dma_start(
                            o_ap[p0 + gp - gn:p0 + gp + gn].rearrange(
                                "p (jt j) b -> j p jt b", j=128
                            ),
                            outf[:],
                        )
                else:
                    eng.dma_start(
                        o_ap[p0 + gp:p0 + gp + gn].rearrange(
                            "p (jt j) b -> j p jt b", j=128
                        ),
                        outf[:],
                    )
                gp += gn


class _Bacc(bacc.Bacc):
    """Bacc whose activation-table pass only considers the one table set
    that holds both Exp and Ln, so there are no mid-kernel table loads
    (1.3us each).  List order/length preserved so act_func_set_id still
    indexes act_info.json correctly."""

    def insert_act_table_loads(self):
        has_activation = any(
            isinstance(i, mybir.InstActivation)
            for b in self.main_func.blocks
            for i in b.instructions
        )
        if not has_activation:
            return
        tables = []
        for name, funcs in get_activation_tables(self.m.arch).items():
            if name != "natural_log_exp_and_others":
                funcs = set()
            tables.append((name, funcs))
        _bass_rust.insert_act_table_loads(self, tables)


def build_nc(pairs=PAIRS_PER_CORE, **opts):
    nc = _Bacc("TRN2", target_bir_lowering=False, debug=False)
    x_d = nc.dram_tensor("xt", [pairs, NUM_IN, BATCH], F16, kind="ExternalInput")
    a_d = nc.dram_tensor("acc", [pairs, NUM_IN, NUM_SUMS], F16, kind="ExternalInput")
    o_d = nc.dram_tensor("out", [pairs, NUM_SUMS, BATCH], F16, kind="ExternalOutput")
    with tile.TileContext(nc) as tc:
        emit_densesum(tc, x_d.ap(), a_d.ap(), o_d.ap(), pairs, **opts)
    nc.compile()
    return nc


_NC_CACHE = {}


def _get_nc():
    key = "main"
    if key not in _NC_CACHE:
        _NC_CACHE[key] = build_nc()
    return _NC_CACHE[key]


def kernel(x: np.ndarray, accumulators: np.ndarray) -> np.ndarray:
    assert x.shape == (NUM_SCOPES, NUM_DECOMPS, BATCH, NUM_IN)
    assert accumulators.shape == (NUM_SCOPES, NUM_DECOMPS, NUM_IN, NUM_SUMS)
    nc = _get_nc()
    # host-side layout prep: x -> x^T[p, i, b] fp16, acc -> fp16
    xt = np.ascontiguousarray(
        np.asarray(x, dtype=np.float32)
        .reshape(NUM_SCOPES * NUM_DECOMPS, BATCH, NUM_IN)
        .swapaxes(1, 2)
        .astype(np.float16)
    )
    a = np.ascontiguousarray(accumulators, dtype=np.float32).astype(np.float16)
    a = a.reshape(NUM_SCOPES * NUM_DECOMPS, NUM_IN, NUM_SUMS)
    in_maps = []
    for c in range(N_CORES):
        q0 = c * PAIRS_PER_CORE
        q1 = q0 + PAIRS_PER_CORE
        in_maps.append({"xt": xt[q0:q1], "acc": a[q0:q1]})
    res = run_bass_kernel_spmd(nc, in_maps, core_ids=list(range(N_CORES)))
    outs = [
        np.asarray(res.results[c]["out"], dtype=np.float32)
        .swapaxes(1, 2)  # [p, j, b] -> [p, b, j]
        .reshape(SCOPES_PER_CORE, NUM_DECOMPS, BATCH, NUM_SUMS)
        for c in range(N_CORES)
    ]
    return np.concatenate(outs, axis=0)

